# revision 13
# baseline (speedup 1.0000x reference)
"""Per-pixel depthwise 3x3 conv (Conv2dLocal) on 8 Trainium2 NeuronCores.

out[b,c,h,w] = sum_{i,j in 3x3} x[b,c,h+i-1,w+j-1] * weight[b, c*9+3i+j, h, w]

Sharding: 8 cores = 2 batches x 4 H-slabs of 64 rows (data/spatial parallel).
The host pads the input spatially (1-px halo on H and W) and hands every core
an overlapping x slab, so the device program is identical and branch-free on
all cores (pure SPMD, no collectives).

The kernel is memory-bound; the rel-err budget (2e-2) is spent on lossy
weight compression: the host downcasts x and 6 of 9 weight taps to fp16 and
3 taps to int8 (q = round(32*w), ~0.9% rel err per tap - all fp16 weights
are scaled by 32 too so every tap shares one scale; the host divides the
output by 32).

Per-core layout: partition p = hb*32 + c (hb: 16-row block 0..3, c: channel);
free dim = (row, w), so all nine 3x3 tap shifts are free-dim offsets into a
single resident x slab [128, 18, 514] (fp16).

Compute: DVE does the 9 per-tap multiplies in fp16 2x_1P mode (both operands
SBUF, unit stride, 4B-aligned). The j=1 column shift is odd, which would
demote the mode - so the HOST shifts those three weight planes right by one
column (into 514-wide planes): the DVE multiply then reads the slab
4B-aligned from column 0 and the PE accumulate un-shifts by reading the
product at columns 1:513 (PE reads have no alignment constraint). PE
accumulates the products per group via exact identity-matmul (fp16 full
rate, fp32 PSUM accumulate, start/stop over the taps); ScalarE downcasts
PSUM->SBUF fp16 and the result streams out.

DMA: x rows 0:6 and the group-0 int8 chunk ride the scalar HWDGE ring,
which starts draining before the weight stream needs it; the fp16 weights
stream tap-by-tap on the sync ring in exactly DVE consumption order, with
the next group's int8 chunk prefetched behind them. ScalarE upcasts each
int8 chunk to fp16 one group ahead of its use.
"""

import sys

if "/opt/trn_rl_repo" not in sys.path:
    sys.path.insert(0, "/opt/trn_rl_repo")

from contextlib import ExitStack

import numpy as np

import concourse.mybir as mybir
import concourse.tile as tile
from concourse import bacc
from concourse.bass_utils import run_bass_kernel_spmd
from concourse.masks import make_identity

# Problem shape (hardcoded per harness contract)
B, C, H, W = 2, 32, 256, 512
K = 3
KK = K * K
N_CORES = 8

# Per-core decomposition
HL = H // 4          # 64 local rows per core
HB = 4               # row-blocks per core (partition groups)
RB = HL // HB        # 16 rows per partition
G = 4                # rows processed per group
NGRP = RB // G       # 4 groups
WP = W + 2           # width incl. halo
NP = 128             # partitions

FP32 = mybir.dt.float32
FP16 = mybir.dt.float16
I8 = mybir.dt.int8
MULT = mybir.AluOpType.mult

SCALE = 32.0
# fp16 taps first in TT order (no upcast dependency - the first weight
# tile lands ~9.7us on the sync ring); int8 taps last so their upcast
# (gated on the scalar-ring int8 chunk) has maximal lead; t = 3*i + j.
# j=1 taps (1, 4, 7) are host-shifted 514-wide planes.
TAP_ORDER = [2, 5, 8, 1, 4, 7, 0, 3, 6]
NI8 = 3                      # TAP_ORDER[-NI8:] stored as int8
F16_ORDER = TAP_ORDER[: KK - NI8]
I8_ORDER = TAP_ORDER[KK - NI8 :]
SHIFTED = {1, 4, 7}
_PROGRAM = None


def _build_program() -> bacc.Bacc:
    nc = bacc.Bacc(
        "TRN2", target_bir_lowering=False, debug=False, num_devices=N_CORES
    )
    x_d = nc.declare_dram_parameter("x", [HB, C, RB + 2, WP], FP16, isOutput=False)
    # fp16 taps in TAP_ORDER[NI8:] order, all padded to 514 wide (the j=1
    # planes shifted right by one column within the pad)
    w_d = nc.declare_dram_parameter(
        "w", [NGRP, KK - NI8, HB, C, G, WP], FP16, isOutput=False
    )
    w8_d = nc.declare_dram_parameter(
        "w8", [NGRP, HB, C, NI8, G, W], I8, isOutput=False
    )
    o_d = nc.declare_dram_parameter("o", [NGRP, HB, C, G, W], FP16, isOutput=True)

    with tile.TileContext(nc) as tc, ExitStack() as ctx:
        x_pool = ctx.enter_context(tc.tile_pool(name="x", bufs=1))
        w_pool = ctx.enter_context(tc.tile_pool(name="wt", bufs=16))
        w8_pool = ctx.enter_context(tc.tile_pool(name="wt8", bufs=2))
        w8f_pool = ctx.enter_context(tc.tile_pool(name="wt8f", bufs=2))
        prod_pool = ctx.enter_context(tc.tile_pool(name="prod", bufs=6))
        out_pool = ctx.enter_context(tc.tile_pool(name="outsb", bufs=2))
        const_pool = ctx.enter_context(tc.tile_pool(name="const", bufs=1))
        pe_pool = ctx.enter_context(tc.tile_pool(name="pe", bufs=2, space="PSUM"))

        ident = const_pool.tile([NP, NP], FP16)
        make_identity(nc, ident)

        # x slab: per partition 18 rows (16 + 2 halo) x 514 cols. Group 0's
        # rows load first for a fast ramp.
        x_sb = x_pool.tile([NP, RB + 2, WP], FP16)
        nc.scalar.dma_start(out=x_sb[:, 0:6, :], in_=x_d[:, :, 0:6, :])

        # int8 chunk DMAs prefetch one group ahead of the fp16 stream, and
        # ScalarE upcasts each to fp16 so every DVE multiply keeps 2x mode.
        def fetch_w8(g, engine):
            wt8 = w8_pool.tile([NP, NI8, G, W], I8, tag="wt8", name=f"wt8_{g}")
            engine.dma_start(out=wt8, in_=w8_d[g])
            return wt8

        w8f = [None] * NGRP

        def upcast_w8(g, wt8):
            w8f[g] = w8f_pool.tile(
                [NP, NI8, G, W], FP16, tag="wt8f", name=f"w8f_{g}"
            )
            nc.scalar.copy(out=w8f[g][:], in_=wt8[:])

        w8_next = fetch_w8(0, nc.scalar)
        nc.scalar.dma_start(
            out=x_sb[:, 6 : RB + 2, :], in_=x_d[:, :, 6 : RB + 2, :]
        )
        # group 0's upcast split per-tap so each int8 tap unblocks early
        w8f[0] = w8f_pool.tile([NP, NI8, G, W], FP16, tag="wt8f", name="w8f_0")
        for t in range(NI8):
            nc.scalar.copy(out=w8f[0][:, t : t + 1], in_=w8_next[:, t : t + 1])
        for grp in range(NGRP):
            R = grp * G
            acc = pe_pool.tile([NP, G, W], FP32, tag="acc")
            w8_cur, w8_next = (
                w8_next,
                fetch_w8(grp + 1, nc.sync) if grp + 1 < NGRP else None,
            )
            for idx, t in enumerate(TAP_ORDER):
                i, j = t // K, t % K
                if idx >= KK - NI8:
                    wt = w8f[grp][:, idx - (KK - NI8)]
                    xin = x_sb[:, R + i : R + i + G, j : j + W]
                    prod = prod_pool.tile([NP, G, W], FP16, tag="prod")
                    nc.vector.tensor_tensor(prod[:], wt, xin, MULT)
                    rd = prod[:, :, :]
                else:
                    wt = w_pool.tile([NP, G, WP], FP16, tag="wt")
                    nc.sync.dma_start(out=wt, in_=w_d[grp, idx])
                    if t in SHIFTED:
                        # j=1: plane pre-shifted right by one column; read
                        # the slab aligned, un-shift in the PE read.
                        xin = x_sb[:, R + i : R + i + G, :]
                        prod = prod_pool.tile([NP, G, WP], FP16, tag="prod14")
                        nc.vector.tensor_tensor(prod[:], wt[:], xin, MULT)
                        rd = prod[:, :, 1 : 1 + W]
                    else:
                        xin = x_sb[:, R + i : R + i + G, j : j + W]
                        prod = prod_pool.tile([NP, G, W], FP16, tag="prod")
                        nc.vector.tensor_tensor(
                            prod[:], wt[:, :, 0:W], xin, MULT
                        )
                        rd = prod[:, :, :]
                # Exact accumulation: ident.T @ prod == prod, summed into
                # fp32 PSUM across the taps (one matmul per PSUM bank).
                for c in range(G):
                    nc.tensor.matmul(
                        acc[:, c, :],
                        ident[:],
                        rd[:, c, :],
                        start=(idx == 0),
                        stop=(idx == KK - 1),
                        skip_group_check=True,
                    )
            if grp + 1 < NGRP:
                upcast_w8(grp + 1, w8_next)
            out_sb = out_pool.tile([NP, G, W], FP16, tag="outsb")
            if grp == NGRP - 1:
                # Pipeline the drain: each row-pair's copy starts as soon as
                # its two PSUM banks hit their stop-matmul, and the first
                # half's store overlaps the second half's copy.
                h = G // 2
                nc.scalar.copy(out=out_sb[:, 0:h, :], in_=acc[:, 0:h, :])
                nc.scalar.dma_start(
                    out=o_d[grp, :, :, 0:h, :], in_=out_sb[:, 0:h, :]
                )
                nc.scalar.copy(out=out_sb[:, h:G, :], in_=acc[:, h:G, :])
                nc.scalar.dma_start(
                    out=o_d[grp, :, :, h:G, :], in_=out_sb[:, h:G, :]
                )
            else:
                nc.scalar.copy(out=out_sb[:], in_=acc[:])
                nc.scalar.dma_start(out=o_d[grp], in_=out_sb[:])

    nc.compile()
    return nc


def _get_program() -> bacc.Bacc:
    global _PROGRAM
    if _PROGRAM is None:
        _PROGRAM = _build_program()
    return _PROGRAM


def _shard_inputs(input: np.ndarray, weight: np.ndarray) -> list[dict]:
    xp = np.pad(input, ((0, 0), (0, 0), (1, 1), (1, 1))).astype(np.float16)
    ws = weight.astype(np.float32) * SCALE
    in_maps = []
    for k in range(N_CORES):
        b, hb = k // 4, k % 4
        h0 = hb * HL
        xs = xp[b, :, h0 : h0 + HL + 2, :]  # [C, 66, WP]
        # x: the HB overlapping 18-row windows -> [HB, C, 18, WP]
        x4 = np.ascontiguousarray(
            np.stack([xs[:, r0 : r0 + RB + 2, :] for r0 in range(0, HL, RB)])
        )
        # weights -> [grp, tap, hb, c, r, w] (taps pre-ordered by
        # TAP_ORDER), contiguous per (grp, tap) so each device DMA reads
        # one linear block; first NI8 taps quantized to int8 from fp32.
        w6 = (
            ws[b]
            .reshape(C, KK, H, W)[:, :, h0 : h0 + HL, :]
            .reshape(C, KK, HB, NGRP, G, W)
            .transpose(3, 1, 2, 0, 4, 5)
        )  # [grp, tap, hb, c, r, w]
        wf = np.zeros((NGRP, KK - NI8, HB, C, G, WP), dtype=np.float16)
        for kk, t in enumerate(F16_ORDER):
            if t in SHIFTED:
                wf[:, kk, :, :, :, 1 : 1 + W] = w6[:, t]
            else:
                wf[:, kk, :, :, :, 0:W] = w6[:, t]
        w8 = np.clip(
            np.rint(w6[:, I8_ORDER].transpose(0, 2, 3, 1, 4, 5)),
            -127,
            127,
        ).astype(np.int8)
        in_maps.append({"x": x4, "w": wf, "w8": w8})
    return in_maps


def kernel(input: np.ndarray, weight: np.ndarray, _trace: bool = False):
    nc = _get_program()
    in_maps = _shard_inputs(np.asarray(input), np.asarray(weight))
    res = run_bass_kernel_spmd(
        nc, in_maps, core_ids=list(range(N_CORES)), trace=_trace
    )
    out = np.empty((B, C, H, W), dtype=np.float32)
    inv = 1.0 / SCALE
    for k in range(N_CORES):
        b, hb = k // 4, k % 4
        # device out [grp, hb, c, r, w] -> [c, hb*16 + grp*4 + r, w]
        o = (
            res.results[k]["o"]
            .reshape(NGRP, HB, C, G, W)
            .transpose(2, 1, 0, 3, 4)
            .reshape(C, HL, W)
            .astype(np.float32)
        )
        out[b, :, hb * HL : (hb + 1) * HL, :] = o * inv
    if _trace:
        return out, res
    return out
